# revision 3
# baseline (speedup 1.0000x reference)
"""Trainium2 Bass kernel for nn_CustomParameterTransform (scatter_memory).

Reference semantics: coord_v [256, 30] holds 10 (x, y, mass) triplets per
sample. Each triplet maps to integer grid indices (x_i, y_i, m_i); a one-hot
volume z [B, 16, 128, 128] is scattered (z[b, m, y, x] = 1) and the output is
concat(1-z, z) over the channel axis -> [256, 32, 128, 128] f32 (512 MB).

Strategy (8 NeuronCores, 32 samples/core, no cross-core comm): the output is
almost entirely constant, so the kernel is a pure HBM-write stream (64 MB per
core) plus 640 one-element fixups per core.

Per-core output layout (host re-assembles): ones region [32 samples x 1 MB]
(the 1-z half: all 1.0 except scatter points), then zeros region (the z half).

All fills are gpsimd SWDGE *indirect* DMAs: each instruction carries 128
descriptor slots; slot r writes one 32 KB block at a host-supplied block
index (int32), or is skipped when the index is out-of-bounds (sem still
fires). Descriptor slot r maps to a fixed DMA engine: rows [4q,4q+4) go to
engine (2q)%16 for q<16 and (2(q-16)+1)%16 otherwise (measured). On this
box one specific engine per even-numbered physical core intermittently runs
~20% slow (nc0/nc4 -> engine position 15, nc2/nc6 -> position 0, max ~207us
vs 172us); the host starves that engine's 8 rows on the affected cores and
redistributes its blocks, which equalizes engine finish times and is free
when the engine happens to be healthy.
"""

import numpy as np

B = 256
NSRC = 10
NMC = 16
L = 128
NCORES = 8
BL = B // NCORES            # 32 samples per core
PLANE = L * L               # 16384
HALF = NMC * PLANE          # 262144 elements per sample half (1 MB)
REGION = BL * HALF          # 8388608 elements per region (32 MB)
OUT_ELEMS = 2 * REGION      # 16777216 per core (64 MB)

BLK = 8192                  # elements per 32 KB fill block
NBLOCKS = OUT_ELEMS // BLK  # 2048
HEAD_UNIT = 1024            # elements per 4 KB head-fill unit
HEAD_UNITS = 128            # head covers 512 KB = first 16 blocks
HEAD_BLOCKS = HEAD_UNITS * HEAD_UNIT // BLK  # 16

N_ONES_MAIN = 9             # 32KB-unit fill instructions for ones region
N_ZEROS_MAIN = 9
N_FILLS = 1 + N_ONES_MAIN + N_ZEROS_MAIN     # 19
N_SCAT = 6                  # 3 columns per region, 128 points each
NCOL = N_FILLS + N_SCAT     # offs input columns

OOB = np.int32(0x7FFFFFF)

# jax core index -> engine position to starve (measured; None = flat).
# jax cores map to physical nc (4,5,6,7,2,3,0,1); even nc cores have one
# intermittently slow engine: nc0,nc4 -> pos 15; nc2,nc6 -> pos 0.
STARVE_POS = {0: 15, 2: 0, 4: 0, 6: 15}
# Byte quota (in 32KB blocks, per region) for a starved engine: chosen so
# slow-rate(21.3GB/s) * quota ~= healthy-rate(26.5GB/s) * healthy share.
SLOW_QUOTA = 52

_CACHE = {}


def _rows_of_pos(p):
    """The 8 descriptor-slot rows served by DMA engine position p."""
    if p % 2 == 0:
        q = p // 2
        return list(range(4 * q, 4 * q + 4)) + \
            list(range(4 * (q + 8), 4 * (q + 8) + 4))
    q = (p - 1) // 2
    return [64 + r for r in range(4 * q, 4 * q + 4)] + \
        [64 + r for r in range(4 * (q + 8), 4 * (q + 8) + 4)]


def _region_layout(starve_pos):
    """Assign a region's 32KB blocks (relative block ids) to (instr, row)
    slots.

    Returns (cols, blk_instr): cols = int32 [128, 9] relative block ids
    (or -1 for OOB), blk_instr = per-block instruction index 0..8.
    The ones region uses relative ids 16..1023 (head covers 0..15); zeros
    uses 0..1023. Call with the relative id list.
    """
    starved = set(_rows_of_pos(starve_pos)) if starve_pos is not None else set()
    healthy_rows = [r for r in range(128) if r not in starved]
    slow_rows = sorted(starved)

    def assign(block_ids):
        cols = np.full((128, 9), -1, dtype=np.int64)
        blk_instr = {}
        it = iter(range(len(block_ids)))
        pos = 0
        quota = SLOW_QUOTA if starved else 0
        for k in range(9):
            rows = list(healthy_rows)
            if quota > 0:
                take = min(8, quota)
                rows += slow_rows[:take]
                quota -= take
            rows.sort()
            for r in rows:
                if pos >= len(block_ids):
                    break
                cols[r, k] = block_ids[pos]
                blk_instr[block_ids[pos]] = k
                pos += 1
            if pos >= len(block_ids):
                break
        assert pos == len(block_ids), (pos, len(block_ids))
        return cols, blk_instr

    return assign


def _class_layouts():
    """Per starve-class: (ones_cols, zeros_cols, ones_map, zeros_map)."""
    out = {}
    for sp in (None, 0, 15):
        assign = _region_layout(sp)
        ones_cols, ones_map = assign(list(range(HEAD_BLOCKS, 1024)))
        zeros_cols, zeros_map = assign(list(range(0, 1024)))
        out[sp] = (ones_cols, zeros_cols, ones_map, zeros_map)
    return out


_LAYOUTS = _class_layouts()


def _col_point_bound(j):
    """Worst-case largest relative block id touched by scatter column j
    (0..2 within a region): points are exactly 10 per sample, sorted by
    address, so point index p belongs to sample p//10."""
    last_pt = min(128 * j + 127, BL * NSRC - 1)
    last_sample = last_pt // 10
    return 32 * (last_sample + 1) - 1  # < 1024


def _scatter_dep_table():
    """Static dep table: scatter col j (global 0..5) -> list of fill
    instruction indices (global 0..18) it must wait for."""
    deps = []
    for j in range(3):  # ones cols
        bmax = _col_point_bound(j)
        last = 0
        for sp, (oc, zc, om, zm) in _LAYOUTS.items():
            ks = [k for b, k in om.items() if b <= bmax]
            last = max(last, max(ks) if ks else 0)
        deps.append([0] + [1 + k for k in range(last + 1)])
    for j in range(3):  # zeros cols
        bmax = _col_point_bound(j)
        last = 0
        for sp, (oc, zc, om, zm) in _LAYOUTS.items():
            ks = [k for b, k in zm.items() if b <= bmax]
            last = max(last, max(ks) if ks else 0)
        deps.append([1 + N_ONES_MAIN + k for k in range(last + 1)])
    return deps


_SCAT_DEPS = _scatter_dep_table()


def _build_nc():
    import concourse.bass as bass
    import concourse.tile as tile
    from concourse import bacc, mybir
    from concourse.tile_rust import add_dep_helper

    import types as _types
    from concourse.vector_clock import ScopedClock

    nc = bacc.Bacc("TRN2", target_bir_lowering=False, debug=False,
                   num_devices=NCORES)

    def _light_drain_and_barrier(self, tick_clock, wait_clock):
        """Replaces TileContext._drain_and_barrier. The stock epilogue is
        drain + two all-engine EVSEM butterfly barriers around the sem clear
        (~9 us). Requirements at kernel end: (1) all DMA completions
        observed, (2) sems cleared for NEFF re-execution, (3) the clear
        happens after every engine's last sem use. (1) is the sync drain's
        global-clock waits; (3) is a counting-sem join; (2) is the ranged
        clear. The second barrier is unnecessary: a re-execution cannot
        start until every engine -- including the clearing gpsimd -- has
        ended."""
        nc_ = self.nc
        drain_inst = nc_.sync.drain()
        wait_clock.add_sem_waits(
            drain_inst.ins, ScopedClock({None: tick_clock.global_clock}))
        join = nc_.alloc_semaphore("tail_join")
        for eng in nc_.engines.values():
            if eng is not nc_.gpsimd:
                eng.sem_inc(join, 1)
        n_other = len(nc_.engines) - 1
        nc_.gpsimd.wait_ge(join, n_other)
        popped = nc_._tile_sem_poison_stack.pop()
        assert popped == self._sem_poison
        sems = list(self.sems.allocated().values())
        nc_.clear_and_free_semaphores(sems + [join])

    offs = nc.dram_tensor("offs", [128, NCOL], mybir.dt.int32,
                          kind="ExternalInput").ap()
    out = nc.dram_tensor("out", [OUT_ELEMS], mybir.dt.float32,
                         kind="ExternalOutput").ap()

    with tile.TileContext(nc) as tc:
        tc._drain_and_barrier = _types.MethodType(_light_drain_and_barrier, tc)
        with tc.tile_pool(name="src", bufs=1) as pool:
            ones_t = pool.tile([128, BLK], mybir.dt.float32)
            zeros_t = pool.tile([128, BLK], mybir.dt.float32)
            # Head columns first so the first fill can start ~0.5us in;
            # zeros tile is only needed ~80us in.
            nc.vector.memset(ones_t[:, 0:HEAD_UNIT], 1.0)
            nc.vector.memset(ones_t[:, HEAD_UNIT:BLK], 1.0)
            nc.vector.memset(zeros_t[:, :], 0.0)

            ot = pool.tile([128, NCOL], mybir.dt.int32)
            nc.sync.dma_start(ot[:, :], offs[:, :])

            def fill(j, in_ap, unit, bound):
                view = bass.AP(out.tensor, 0, [[unit, 1], [1, unit]],
                               dep_tracking_offset=j * BLK)
                return nc.gpsimd.indirect_dma_start(
                    out=view,
                    out_offset=bass.IndirectOffsetOnAxis(
                        ap=ot[:, j:j + 1], axis=0),
                    in_=in_ap, in_offset=None,
                    bounds_check=bound, oob_is_err=False)

            fills = []
            # instr 0: ones head, 4KB units
            fills.append(fill(0, ones_t[:, 0:HEAD_UNIT], HEAD_UNIT,
                              OUT_ELEMS // HEAD_UNIT - 1))
            for k in range(N_ONES_MAIN):
                fills.append(fill(1 + k, ones_t[:, :], BLK, NBLOCKS - 1))
            for k in range(N_ZEROS_MAIN):
                fills.append(fill(1 + N_ONES_MAIN + k, zeros_t[:, :], BLK,
                                  NBLOCKS - 1))

            # Scatter values: ones-region fixups write 0.0, zeros-region 1.0.
            vals = pool.tile([128, N_SCAT], mybir.dt.float32)
            nc.gpsimd.memset(vals[:, 0:3], 0.0)
            nc.gpsimd.memset(vals[:, 3:6], 1.0)

            for j in range(N_SCAT):
                view = bass.AP(out.tensor, 0, [[1, 1], [1, 1]],
                               dep_tracking_offset=N_FILLS * BLK + j)
                sc = nc.gpsimd.indirect_dma_start(
                    out=view,
                    out_offset=bass.IndirectOffsetOnAxis(
                        ap=ot[:, N_FILLS + j:N_FILLS + j + 1], axis=0),
                    in_=vals[:, j:j + 1], in_offset=None,
                    bounds_check=OUT_ELEMS - 1, oob_is_err=False)
                for fi in _SCAT_DEPS[j]:
                    add_dep_helper(sc.ins, fills[fi].ins,
                                   reason="scatter after covering fills")

    nc.compile()
    return nc


def _compute_indices(coord_v, lows, highs, nmc, L_):
    """Replicates reference.py exactly (same jax ops on the default device)
    so the floor/log10 bin boundaries match bit-for-bit."""
    import jax.numpy as jnp

    cv = jnp.asarray(np.asarray(coord_v, dtype=np.float32))
    n = cv.shape[1] // 3
    v10 = cv.at[:, 2::3].set(jnp.log10(cv[:, 2::3]))
    lo = jnp.tile(jnp.asarray(np.asarray(lows, dtype=np.float32)), n)
    hi = jnp.tile(jnp.asarray(np.asarray(highs, dtype=np.float32)), n)
    coord_grid = (v10 - lo) / (hi - lo)
    tr = coord_grid.reshape(-1, 3)
    x_i = jnp.floor(tr[:, 0] * L_).astype(jnp.int32)
    y_i = jnp.floor(tr[:, 1] * L_).astype(jnp.int32)
    m_i = jnp.floor(tr[:, 2] * nmc).astype(jnp.int32)
    return (np.asarray(x_i), np.asarray(y_i), np.asarray(m_i))


def _prepare_in_maps(coord_v, lows, highs, nmc, L):
    nmc = int(nmc)
    L_ = int(L)
    x_i, y_i, m_i = _compute_indices(coord_v, lows, highs, nmc, L_)
    n_batch = coord_v.shape[0]
    n = coord_v.shape[1] // 3
    b_i = np.repeat(np.arange(n_batch, dtype=np.int64), n)

    # Per-sample-half element offsets in the per-core region layout.
    s_local = (b_i % BL).astype(np.int64)
    off_in_half = (m_i.astype(np.int64) * PLANE
                   + y_i.astype(np.int64) * L_ + x_i.astype(np.int64))
    ones_off = s_local * HALF + off_in_half          # within ones region
    zeros_off = REGION + ones_off                    # within zeros region

    in_maps = []
    pts_per_core = BL * n  # 320
    for c in range(NCORES):
        sp = STARVE_POS.get(c)
        ones_cols, zeros_cols, ones_map, zeros_map = _LAYOUTS[sp]
        offs_np = np.full((128, NCOL), OOB, dtype=np.int32)
        # fill instr 0: ones head, 4KB units 0..127 (first 16 blocks)
        offs_np[:, 0] = np.arange(HEAD_UNITS, dtype=np.int32)
        # ones main fills: relative block ids are absolute here
        m = ones_cols >= 0
        offs_np[:, 1:1 + N_ONES_MAIN][m] = ones_cols[m].astype(np.int32)
        # zeros main fills: shift by region base (in blocks)
        m = zeros_cols >= 0
        offs_np[:, 1 + N_ONES_MAIN:N_FILLS][m] = \
            (zeros_cols[m] + REGION // BLK).astype(np.int32)

        sel = slice(c * pts_per_core, (c + 1) * pts_per_core)
        po = np.sort(ones_off[sel])
        pz = np.sort(zeros_off[sel])
        for j in range(3):
            colp = po[128 * j:128 * j + 128]
            col = np.full(128, po[0], dtype=np.int64)
            col[:len(colp)] = colp
            offs_np[:, N_FILLS + j] = col.astype(np.int32)
            # static dep check: every point's block must be covered by the
            # fill instructions this column waits on
            blocks = colp // BLK
            for bb in blocks:
                if bb >= HEAD_BLOCKS:
                    k = ones_map[int(bb)]
                    assert 1 + k in _SCAT_DEPS[j], (c, j, bb, k)
            colz = pz[128 * j:128 * j + 128]
            col = np.full(128, pz[0], dtype=np.int64)
            col[:len(colz)] = colz
            offs_np[:, N_FILLS + 3 + j] = col.astype(np.int32)
            for bb in (colz - REGION) // BLK:
                k = zeros_map[int(bb)]
                assert 1 + N_ONES_MAIN + k in _SCAT_DEPS[3 + j], (c, j, bb, k)
        in_maps.append({"offs": offs_np})
    return in_maps


def _run(in_maps, **kwargs):
    if "nc" not in _CACHE:
        _CACHE["nc"] = _build_nc()
    nc = _CACHE["nc"]
    from concourse.bass_utils import run_bass_kernel_spmd
    return run_bass_kernel_spmd(nc, in_maps, core_ids=list(range(NCORES)),
                                **kwargs)


def kernel(coord_v, lows, highs, nmc, L):
    nmc = int(nmc)
    L_ = int(L)
    assert nmc == NMC and L_ == globals()["L"], (nmc, L_)

    in_maps = _prepare_in_maps(coord_v, lows, highs, nmc, L_)
    res = _run(in_maps)
    parts = []
    for c in range(NCORES):
        o = res.results[c]["out"]
        ones = o[0:REGION].reshape(BL, NMC, L_, L_)
        zeros = o[REGION:].reshape(BL, NMC, L_, L_)
        parts.append(np.concatenate((ones, zeros), axis=1))
    return np.concatenate(parts, axis=0)


# revision 5
# speedup vs baseline: 1.1609x; 1.1609x over previous
"""Trainium2 Bass kernel for nn_CustomParameterTransform (scatter_memory).

Reference semantics: coord_v [256, 30] holds 10 (x, y, mass) triplets per
sample. Each triplet maps to integer grid indices (x_i, y_i, m_i); a one-hot
volume z [B, 16, 128, 128] is scattered (z[b, m, y, x] = 1) and the output is
concat(1-z, z) over the channel axis -> [256, 32, 128, 128] f32 (512 MB).

Strategy (8 NeuronCores, 32 samples/core, no cross-core comm): the output is
almost entirely constant, so the kernel is a pure HBM write stream (64 MB
per core) plus 640 one-element fixups per core.

Per-core output layout (host re-assembles): ones region [32 samples x 1 MB]
(the 1-z half: 1.0 except scatter points), then zeros region (the z half).

Fill plan, 2048 32KB blocks per core:
  - 52 MB static HWDGE fills (sync: most of ones; scalar: most of zeros)
    from constant SBUF tiles - every DMA engine gets exactly 104 blocks.
  - 12.5 MB early SWDGE indirect fills (gpsimd) whose 32KB blocks are
    addressed by a host-supplied per-core index tensor; descriptor slot
    rows map to fixed DMA engines (rows [4q,4q+4) -> engine (2q)%16 for
    q<16, else (2(q-16)+1)%16 - measured), and out-of-bounds indices are
    silently skipped, so the host shapes per-engine bytes per core.
  On this box one specific engine per even-numbered physical core
  intermittently runs ~20% slow (nc0/nc4 -> engine position 15, nc2/nc6 ->
  position 0; jax cores map to nc (4,5,6,7,2,3,0,1)). 104 blocks is the
  optimal share for a slow engine (104/21.3GB/s ~= 129.5/26.5GB/s), so on
  risky cores the host gives that engine no SWDGE blocks at all and spreads
  them over the other 15 engines; on healthy cores the layout is flat.
  Equalized finish ~157us vs ~197us for a flat layout with a slow engine,
  and the skew costs nothing when the engine is healthy.
  - 640 scatter fixups as 6 indirect-DMA columns (ones cols write 0.0,
    zeros cols 1.0), each depending only on the fills covering its
    address range so the last one fires right after the final fill.
"""

import numpy as np

B = 256
NSRC = 10
NMC = 16
L = 128
NCORES = 8
BL = B // NCORES            # 32 samples per core
PLANE = L * L               # 16384
HALF = NMC * PLANE          # 262144 elements per sample half (1 MB)
REGION = BL * HALF          # 8388608 elements per region (32 MB)
OUT_ELEMS = 2 * REGION      # 16777216 per core (64 MB)

BLK = 8192                  # elements per 32 KB fill block
NBLOCKS = OUT_ELEMS // BLK  # 2048
ZBASE = REGION // BLK       # first zeros-region block (1024)
HEAD_UNIT = 1024            # elements per 4 KB head-fill unit
HEAD_BLOCKS = 16            # head covers 512 KB = blocks 0..15

# SWDGE-shaped block ranges (per region, absolute block ids)
ONES_SH_LO, ONES_SH_HI = HEAD_BLOCKS, 192        # 176 blocks
ZEROS_SH_LO, ZEROS_SH_HI = ZBASE, ZBASE + 192    # 192 blocks
N_SH_ONES = 2               # shaped instructions per region
N_SH_ZEROS = 2
N_SW_FILLS = 1 + N_SH_ONES + N_SH_ZEROS          # 5 SWDGE fill instrs

# Static HWDGE fills: (queue, start block, nblocks); R2 = stride-0 repeat.
SYNC_FILLS = [(192, 256), (448, 256), (704, 256), (1984, 64)]
SCAL_FILLS = [(960, 64), (1216, 256), (1472, 256), (1728, 256)]

N_SCAT = 6
NCOL = N_SW_FILLS + N_SCAT  # offs input columns

OOB = np.int32(0x7FFFFFF)

# jax core index -> engine position to starve (measured; absent = flat).
STARVE_POS = {0: 15, 2: 0, 4: 0, 6: 15}

_CACHE = {}


def _rows_of_pos(p):
    """The 8 descriptor-slot rows served by DMA engine position p."""
    if p % 2 == 0:
        q = p // 2
        return list(range(4 * q, 4 * q + 4)) + \
            list(range(4 * (q + 8), 4 * (q + 8) + 4))
    q = (p - 1) // 2
    return [64 + r for r in range(4 * q, 4 * q + 4)] + \
        [64 + r for r in range(4 * (q + 8), 4 * (q + 8) + 4)]


_POS_ROWS = [_rows_of_pos(p) for p in range(16)]


def _shaped_cols(starve_pos, block_ids, n_instr):
    """Assign shaped blocks to (instr, row) slots, engine-balanced.

    Returns int64 [128, n_instr] block ids (-1 = OOB slot)."""
    nb = len(block_ids)
    quota = [0] * 16
    if starve_pos is None:
        for p in range(16):
            quota[p] = nb // 16
        for p in range(nb % 16):
            quota[p] += 1
    else:
        healthy = [p for p in range(16) if p != starve_pos]
        for i, p in enumerate(healthy):
            quota[p] = nb // 15 + (1 if i < nb % 15 else 0)
    cols = np.full((128, n_instr), -1, dtype=np.int64)
    pos_it = 0
    for p in range(16):
        rows = _POS_ROWS[p]
        q = quota[p]
        assert q <= 8 * n_instr, (p, q)
        left = q
        for k in range(n_instr):
            t = min(8, left)
            for r in rows[:t]:
                cols[r, k] = block_ids[pos_it]
                pos_it += 1
            left -= t
    assert pos_it == nb, (pos_it, nb)
    return cols


def _class_layouts():
    out = {}
    for sp in (None, 0, 15):
        oc = _shaped_cols(sp, list(range(ONES_SH_LO, ONES_SH_HI)), N_SH_ONES)
        zc = _shaped_cols(sp, list(range(ZEROS_SH_LO, ZEROS_SH_HI)),
                          N_SH_ZEROS)
        out[sp] = (oc, zc)
    return out


_LAYOUTS = _class_layouts()


def _col_bmax(j):
    """Worst-case largest region-relative block id touched by scatter
    column j (0..2): points are exactly 10 per sample, address-sorted."""
    last_pt = min(128 * j + 127, BL * NSRC - 1)
    return 32 * (last_pt // 10 + 1) - 1


def _build_nc():
    import concourse.bass as bass
    import concourse.tile as tile
    from concourse import bacc, mybir
    from concourse.tile_rust import add_dep_helper

    import types as _types
    from concourse.vector_clock import ScopedClock

    nc = bacc.Bacc("TRN2", target_bir_lowering=False, debug=False,
                   num_devices=NCORES)

    def _light_drain_and_barrier(self, tick_clock, wait_clock):
        """Replaces TileContext._drain_and_barrier. The stock epilogue is
        drain + two all-engine EVSEM butterfly barriers around the sem
        clear (~9 us). Requirements at kernel end: (1) all DMA completions
        observed, (2) sems cleared for NEFF re-execution, (3) the clear
        after every engine's last sem use. (1) is the sync drain's
        global-clock waits; (3) is a counting-sem join; (2) the ranged
        clear. The second barrier is unnecessary: a re-execution cannot
        start until every engine - including the clearing gpsimd - has
        ended."""
        nc_ = self.nc
        drain_inst = nc_.sync.drain()
        wait_clock.add_sem_waits(
            drain_inst.ins, ScopedClock({None: tick_clock.global_clock}))
        join = nc_.alloc_semaphore("tail_join")
        for eng in nc_.engines.values():
            if eng is not nc_.gpsimd:
                eng.sem_inc(join, 1)
        n_other = len(nc_.engines) - 1
        nc_.gpsimd.wait_ge(join, n_other)
        popped = nc_._tile_sem_poison_stack.pop()
        assert popped == self._sem_poison
        sems = list(self.sems.allocated().values())
        nc_.clear_and_free_semaphores(sems + [join])

    offs = nc.dram_tensor("offs", [128, NCOL], mybir.dt.int32,
                          kind="ExternalInput").ap()
    out = nc.dram_tensor("out", [OUT_ELEMS], mybir.dt.float32,
                         kind="ExternalOutput").ap()

    with tile.TileContext(nc) as tc:
        tc._drain_and_barrier = _types.MethodType(_light_drain_and_barrier, tc)
        with tc.tile_pool(name="src", bufs=1) as pool:
            ones_t = pool.tile([128, BLK], mybir.dt.float32)
            zeros_t = pool.tile([128, BLK], mybir.dt.float32)
            # head columns first so the first SWDGE fill starts ~1us in
            nc.vector.memset(ones_t[:, 0:HEAD_UNIT], 1.0)
            nc.vector.memset(ones_t[:, HEAD_UNIT:BLK], 1.0)
            nc.vector.memset(zeros_t[:, :], 0.0)

            ot = pool.tile([128, NCOL], mybir.dt.int32)
            nc.sync.dma_start(ot[:, :], offs[:, :])

            def swfill(j, in_ap, unit, bound):
                view = bass.AP(out.tensor, 0, [[unit, 1], [1, unit]],
                               dep_tracking_offset=j * BLK)
                return nc.gpsimd.indirect_dma_start(
                    out=view,
                    out_offset=bass.IndirectOffsetOnAxis(
                        ap=ot[:, j:j + 1], axis=0),
                    in_=in_ap, in_offset=None,
                    bounds_check=bound, oob_is_err=False)

            sw = []
            sw.append(swfill(0, ones_t[:, 0:HEAD_UNIT], HEAD_UNIT,
                             OUT_ELEMS // HEAD_UNIT - 1))
            for k in range(N_SH_ONES):
                sw.append(swfill(1 + k, ones_t[:, :], BLK, NBLOCKS - 1))
            for k in range(N_SH_ZEROS):
                sw.append(swfill(1 + N_SH_ONES + k, zeros_t[:, :], BLK,
                                 NBLOCKS - 1))

            ones_r2 = bass.AP(ones_t[:, :].tensor, 0,
                              [[BLK, 128], [0, 2], [1, BLK]])
            zeros_r2 = bass.AP(zeros_t[:, :].tensor, 0,
                               [[BLK, 128], [0, 2], [1, BLK]])

            def static_fill(eng, start, nblk, tile_full, tile_r2):
                a, b = start * BLK, (start + nblk) * BLK
                if nblk == 256:
                    return eng.dma_start(out[a:b], tile_r2)
                assert nblk == 64
                return eng.dma_start(out[a:b], tile_full[:, 0:4096])

            sync_f = [static_fill(nc.sync, s, n,
                                  ones_t if s < ZBASE else zeros_t,
                                  ones_r2 if s < ZBASE else zeros_r2)
                      for s, n in SYNC_FILLS]
            scal_f = [static_fill(nc.scalar, s, n,
                                  ones_t if s < ZBASE else zeros_t,
                                  ones_r2 if s < ZBASE else zeros_r2)
                      for s, n in SCAL_FILLS]

            statics = {s: f for (s, n), f in
                       zip(SYNC_FILLS, sync_f)} | \
                      {s: f for (s, n), f in zip(SCAL_FILLS, scal_f)}

            def covering(bmax_abs, lo_abs):
                """Static fills intersecting blocks [lo_abs, bmax_abs]."""
                res = []
                for (s, n) in SYNC_FILLS + SCAL_FILLS:
                    if s <= bmax_abs and s + n > lo_abs:
                        res.append(statics[s])
                return res

            vals = pool.tile([128, N_SCAT], mybir.dt.float32)
            nc.gpsimd.memset(vals[:, 0:3], 0.0)
            nc.gpsimd.memset(vals[:, 3:6], 1.0)

            for j in range(N_SCAT):
                view = bass.AP(out.tensor, 0, [[1, 1], [1, 1]],
                               dep_tracking_offset=50000 + j)
                sc = nc.gpsimd.indirect_dma_start(
                    out=view,
                    out_offset=bass.IndirectOffsetOnAxis(
                        ap=ot[:, N_SW_FILLS + j:N_SW_FILLS + j + 1], axis=0),
                    in_=vals[:, j:j + 1], in_offset=None,
                    bounds_check=OUT_ELEMS - 1, oob_is_err=False)
                if j < 3:
                    bmax = _col_bmax(j)
                    deps = sw[0:1 + N_SH_ONES] + covering(bmax, 0)
                else:
                    bmax = ZBASE + _col_bmax(j - 3)
                    deps = sw[1 + N_SH_ONES:] + covering(bmax, ZBASE)
                for f in deps:
                    add_dep_helper(sc.ins, f.ins,
                                   reason="scatter after covering fills")

    nc.compile()
    return nc


def _compute_indices(coord_v, lows, highs, nmc, L_):
    """Replicates reference.py exactly (same jax ops on the default device)
    so the floor/log10 bin boundaries match bit-for-bit."""
    import jax.numpy as jnp

    cv = jnp.asarray(np.asarray(coord_v, dtype=np.float32))
    n = cv.shape[1] // 3
    v10 = cv.at[:, 2::3].set(jnp.log10(cv[:, 2::3]))
    lo = jnp.tile(jnp.asarray(np.asarray(lows, dtype=np.float32)), n)
    hi = jnp.tile(jnp.asarray(np.asarray(highs, dtype=np.float32)), n)
    coord_grid = (v10 - lo) / (hi - lo)
    tr = coord_grid.reshape(-1, 3)
    x_i = jnp.floor(tr[:, 0] * L_).astype(jnp.int32)
    y_i = jnp.floor(tr[:, 1] * L_).astype(jnp.int32)
    m_i = jnp.floor(tr[:, 2] * nmc).astype(jnp.int32)
    return (np.asarray(x_i), np.asarray(y_i), np.asarray(m_i))


def _prepare_in_maps(coord_v, lows, highs, nmc, L):
    nmc = int(nmc)
    L_ = int(L)
    x_i, y_i, m_i = _compute_indices(coord_v, lows, highs, nmc, L_)
    n_batch = coord_v.shape[0]
    n = coord_v.shape[1] // 3
    b_i = np.repeat(np.arange(n_batch, dtype=np.int64), n)

    s_local = (b_i % BL).astype(np.int64)
    off_in_half = (m_i.astype(np.int64) * PLANE
                   + y_i.astype(np.int64) * L_ + x_i.astype(np.int64))
    ones_off = s_local * HALF + off_in_half
    zeros_off = REGION + ones_off

    in_maps = []
    pts_per_core = BL * n  # 320
    for c in range(NCORES):
        oc, zc = _LAYOUTS[STARVE_POS.get(c)]
        offs_np = np.full((128, NCOL), OOB, dtype=np.int32)
        offs_np[:, 0] = np.arange(128, dtype=np.int32)  # head, 4KB units
        m = oc >= 0
        offs_np[:, 1:1 + N_SH_ONES][m] = oc[m].astype(np.int32)
        m = zc >= 0
        offs_np[:, 1 + N_SH_ONES:N_SW_FILLS][m] = zc[m].astype(np.int32)

        sel = slice(c * pts_per_core, (c + 1) * pts_per_core)
        po = np.sort(ones_off[sel])
        pz = np.sort(zeros_off[sel])
        for j in range(3):
            colp = po[128 * j:128 * j + 128]
            col = np.full(128, po[0], dtype=np.int64)
            col[:len(colp)] = colp
            offs_np[:, N_SW_FILLS + j] = col.astype(np.int32)
            assert (colp // BLK).max(initial=0) <= _col_bmax(j)
            colz = pz[128 * j:128 * j + 128]
            col = np.full(128, pz[0], dtype=np.int64)
            col[:len(colz)] = colz
            offs_np[:, N_SW_FILLS + 3 + j] = col.astype(np.int32)
            assert (colz // BLK).max(initial=0) <= ZBASE + _col_bmax(j)
        in_maps.append({"offs": offs_np})
    return in_maps


def _run(in_maps, **kwargs):
    if "nc" not in _CACHE:
        _CACHE["nc"] = _build_nc()
    nc = _CACHE["nc"]
    from concourse.bass_utils import run_bass_kernel_spmd
    return run_bass_kernel_spmd(nc, in_maps, core_ids=list(range(NCORES)),
                                **kwargs)


def kernel(coord_v, lows, highs, nmc, L):
    nmc = int(nmc)
    L_ = int(L)
    assert nmc == NMC and L_ == globals()["L"], (nmc, L_)

    in_maps = _prepare_in_maps(coord_v, lows, highs, nmc, L_)
    res = _run(in_maps)
    parts = []
    for c in range(NCORES):
        o = res.results[c]["out"]
        ones = o[0:REGION].reshape(BL, NMC, L_, L_)
        zeros = o[REGION:].reshape(BL, NMC, L_, L_)
        parts.append(np.concatenate((ones, zeros), axis=1))
    return np.concatenate(parts, axis=0)


# revision 9
# speedup vs baseline: 1.1877x; 1.0231x over previous
"""Trainium2 Bass kernel for nn_CustomParameterTransform (scatter_memory).

Reference semantics: coord_v [256, 30] holds 10 (x, y, mass) triplets per
sample. Each triplet maps to integer grid indices (x_i, y_i, m_i); a one-hot
volume z [B, 16, 128, 128] is scattered (z[b, m, y, x] = 1) and the output is
concat(1-z, z) over the channel axis -> [256, 32, 128, 128] f32 (512 MB).

Strategy (8 NeuronCores, 32 samples/core, no cross-core comm): the output is
almost entirely constant, so the kernel is a pure HBM write stream (64 MB
per core) plus 640 one-element fixups per core.

Per-core output layout (host re-assembles): ones region [32 samples x 1 MB]
(the 1-z half: 1.0 except scatter points), then zeros region (the z half).

Fill plan, 2048 32KB blocks per core:
  - 52 MB static HWDGE fills (sync: most of ones; scalar: most of zeros)
    from constant SBUF tiles - every DMA engine gets exactly 104 blocks.
  - 12.5 MB early SWDGE indirect fills (gpsimd) whose 32KB blocks are
    addressed by a host-supplied per-core index tensor; descriptor slot
    rows map to fixed DMA engines (rows [4q,4q+4) -> engine (2q)%16 for
    q<16, else (2(q-16)+1)%16 - measured), and out-of-bounds indices are
    silently skipped, so the host shapes per-engine bytes per core.
  On this box one specific engine per even-numbered physical core
  intermittently runs ~20% slow (nc0/nc4 -> engine position 15, nc2/nc6 ->
  position 0; jax cores map to nc (4,5,6,7,2,3,0,1)). 104 blocks is the
  optimal share for a slow engine (104/21.3GB/s ~= 129.5/26.5GB/s), so on
  risky cores the host gives that engine no SWDGE blocks at all and spreads
  them over the other 15 engines; on healthy cores the layout is flat.
  Equalized finish ~157us vs ~197us for a flat layout with a slow engine,
  and the skew costs nothing when the engine is healthy.
  - 640 scatter fixups as 6 indirect-DMA columns (ones cols write 0.0,
    zeros cols 1.0), each depending only on the fills covering its
    address range so the last one fires right after the final fill.
"""

import numpy as np

B = 256
NSRC = 10
NMC = 16
L = 128
NCORES = 8
BL = B // NCORES            # 32 samples per core
PLANE = L * L               # 16384
HALF = NMC * PLANE          # 262144 elements per sample half (1 MB)
REGION = BL * HALF          # 8388608 elements per region (32 MB)
OUT_ELEMS = 2 * REGION      # 16777216 per core (64 MB)

BLK = 8192                  # elements per 32 KB fill block
NBLOCKS = OUT_ELEMS // BLK  # 2048
ZBASE = REGION // BLK       # first zeros-region block (1024)
HEAD_UNIT = 1024            # elements per 4 KB head-fill unit
HEAD_BLOCKS = 16            # head covers 512 KB = blocks 0..15

# SWDGE-shaped block ranges (absolute block ids). The zeros region is
# filled FIRST (its head bootstraps the stream; its memset runs first) and
# the ones region LAST, so five of the six scatter columns fire mid-stream
# and only ones-col2 trails the final fill.
ONES_SH_LO, ONES_SH_HI = 0, 208                  # 208 blocks
ZEROS_SH_LO, ZEROS_SH_HI = ZBASE + 16, ZBASE + 208   # 192 blocks
HEAD_START = ZBASE          # zeros head: blocks 1024..1039
N_SH_ONES = 2               # shaped instructions per region
N_SH_ZEROS = 2
N_SW_FILLS = 1 + N_SH_ONES + N_SH_ZEROS          # 5 SWDGE fill instrs

# Static HWDGE fills (start block, nblocks), in issue order per queue:
# both queues stream zeros first, then ones; the ones tails are the very
# last fills so only scatter col 2 depends on the stream end.
SYNC_FILLS = [(1232, 128), (1360, 128), (1488, 128), (1616, 128),
              (720, 128), (848, 128), (976, 48)]
SCAL_FILLS = [(1744, 128), (1872, 128), (2000, 48),
              (208, 128), (336, 128), (464, 128), (592, 128)]

N_SCAT = 6
NCOL = N_SW_FILLS + N_SCAT  # offs input columns

OOB = np.int32(0x7FFFFFF)

# jax core index -> engine position to starve (measured; absent = flat).
STARVE_POS = {0: 15, 2: 0, 4: 0, 6: 15}

_CACHE = {}


def _rows_of_pos(p):
    """The 8 descriptor-slot rows served by DMA engine position p."""
    if p % 2 == 0:
        q = p // 2
        return list(range(4 * q, 4 * q + 4)) + \
            list(range(4 * (q + 8), 4 * (q + 8) + 4))
    q = (p - 1) // 2
    return [64 + r for r in range(4 * q, 4 * q + 4)] + \
        [64 + r for r in range(4 * (q + 8), 4 * (q + 8) + 4)]


_POS_ROWS = [_rows_of_pos(p) for p in range(16)]


def _shaped_cols(starve_pos, block_ids, n_instr):
    """Assign shaped blocks to (instr, row) slots, engine-balanced.

    Returns int64 [128, n_instr] block ids (-1 = OOB slot)."""
    nb = len(block_ids)
    quota = [0] * 16
    if starve_pos is None:
        for p in range(16):
            quota[p] = nb // 16
        for p in range(nb % 16):
            quota[p] += 1
    else:
        healthy = [p for p in range(16) if p != starve_pos]
        for i, p in enumerate(healthy):
            quota[p] = nb // 15 + (1 if i < nb % 15 else 0)
    cols = np.full((128, n_instr), -1, dtype=np.int64)
    pos_it = 0
    for p in range(16):
        rows = _POS_ROWS[p]
        q = quota[p]
        assert q <= 8 * n_instr, (p, q)
        left = q
        for k in range(n_instr):
            t = min(8, left)
            for r in rows[:t]:
                cols[r, k] = block_ids[pos_it]
                pos_it += 1
            left -= t
    assert pos_it == nb, (pos_it, nb)
    return cols


def _class_layouts():
    out = {}
    for sp in (None, 0, 15):
        oc = _shaped_cols(sp, list(range(ONES_SH_LO, ONES_SH_HI)), N_SH_ONES)
        zc = _shaped_cols(sp, list(range(ZEROS_SH_LO, ZEROS_SH_HI)),
                          N_SH_ZEROS)
        out[sp] = (oc, zc)
    return out


_LAYOUTS = _class_layouts()


def _col_bmax(j):
    """Worst-case largest region-relative block id touched by scatter
    column j (0..2): points are exactly 10 per sample, address-sorted."""
    last_pt = min(128 * j + 127, BL * NSRC - 1)
    return 32 * (last_pt // 10 + 1) - 1


def _build_nc():
    import concourse.bass as bass
    import concourse.tile as tile
    from concourse import bacc, mybir
    from concourse.tile_rust import add_dep_helper

    import types as _types
    from concourse.vector_clock import ScopedClock

    nc = bacc.Bacc("TRN2", target_bir_lowering=False, debug=False,
                   num_devices=NCORES)

    def _light_drain_and_barrier(self, tick_clock, wait_clock):
        """Replaces TileContext._drain_and_barrier. The stock epilogue is
        drain + two all-engine EVSEM butterfly barriers around the sem
        clear (~9 us). Requirements at kernel end: (1) all DMA completions
        observed, (2) sems cleared for NEFF re-execution, (3) the clear
        after every engine's last sem use. (1) is the sync drain's
        global-clock waits; (3) is a counting-sem join; (2) the ranged
        clear. The second barrier is unnecessary: a re-execution cannot
        start until every engine - including the clearing gpsimd - has
        ended."""
        nc_ = self.nc
        drain_inst = nc_.sync.drain()
        wait_clock.add_sem_waits(
            drain_inst.ins, ScopedClock({None: tick_clock.global_clock}))
        join = nc_.alloc_semaphore("tail_join")
        for eng in nc_.engines.values():
            if eng is not nc_.gpsimd:
                eng.sem_inc(join, 1)
        n_other = len(nc_.engines) - 1
        nc_.gpsimd.wait_ge(join, n_other)
        popped = nc_._tile_sem_poison_stack.pop()
        assert popped == self._sem_poison
        sems = list(self.sems.allocated().values())
        nc_.clear_and_free_semaphores(sems + [join])

    offs = nc.dram_tensor("offs", [128, NCOL], mybir.dt.int32,
                          kind="ExternalInput").ap()
    out = nc.dram_tensor("out", [OUT_ELEMS], mybir.dt.float32,
                         kind="ExternalOutput").ap()

    with tile.TileContext(nc) as tc:
        tc._drain_and_barrier = _types.MethodType(_light_drain_and_barrier, tc)
        with tc.tile_pool(name="src", bufs=1) as pool:
            zeros_t = pool.tile([128, BLK], mybir.dt.float32)
            ones_t = pool.tile([128, BLK], mybir.dt.float32)
            # zeros head columns first: the zeros head fill starts ~1us in;
            # ones statics only run in the second half of the stream.
            nc.vector.memset(zeros_t[:, 0:HEAD_UNIT], 0.0)
            nc.vector.memset(zeros_t[:, HEAD_UNIT:BLK], 0.0)
            nc.vector.memset(ones_t[:, :], 1.0)

            ot = pool.tile([128, NCOL], mybir.dt.int32)
            nc.sync.dma_start(ot[:, :], offs[:, :])

            def swfill(j, in_ap, unit, bound):
                view = bass.AP(out.tensor, 0, [[unit, 1], [1, unit]],
                               dep_tracking_offset=j * BLK)
                return nc.gpsimd.indirect_dma_start(
                    out=view,
                    out_offset=bass.IndirectOffsetOnAxis(
                        ap=ot[:, j:j + 1], axis=0),
                    in_=in_ap, in_offset=None,
                    bounds_check=bound, oob_is_err=False)

            sw = [None] * N_SW_FILLS
            sw[0] = swfill(0, zeros_t[:, 0:HEAD_UNIT], HEAD_UNIT,
                           OUT_ELEMS // HEAD_UNIT - 1)
            for k in range(N_SH_ZEROS):
                sw[1 + N_SH_ONES + k] = swfill(
                    1 + N_SH_ONES + k, zeros_t[:, :], BLK, NBLOCKS - 1)

            def static_fill(eng, start, nblk):
                a, b = start * BLK, (start + nblk) * BLK
                t = ones_t if start < ZBASE else zeros_t
                return eng.dma_start(out[a:b], t[:, 0:nblk * BLK // 128])

            sync_f = [static_fill(nc.sync, s, n) for s, n in SYNC_FILLS]
            scal_f = [static_fill(nc.scalar, s, n) for s, n in SCAL_FILLS]

            # vals memsets before the ones-shaped gens: gpsimd stalls on
            # the ones_t memset at that point anyway.
            vals = pool.tile([128, N_SCAT], mybir.dt.float32)
            nc.gpsimd.memset(vals[:, 0:3], 0.0)
            nc.gpsimd.memset(vals[:, 3:6], 1.0)

            for k in range(N_SH_ONES):
                sw[1 + k] = swfill(1 + k, ones_t[:, :], BLK, NBLOCKS - 1)

            statics = {s: f for (s, n), f in
                       zip(SYNC_FILLS, sync_f)} | \
                      {s: f for (s, n), f in zip(SCAL_FILLS, scal_f)}

            def covering(bmax_abs, lo_abs):
                """Static fills intersecting blocks [lo_abs, bmax_abs]."""
                res = []
                for (s, n) in SYNC_FILLS + SCAL_FILLS:
                    if s <= bmax_abs and s + n > lo_abs:
                        res.append(statics[s])
                return res

            # scatters in expected firing order (zeros cols fire mid-stream)
            for j in (3, 4, 5, 0, 1, 2):
                view = bass.AP(out.tensor, 0, [[1, 1], [1, 1]],
                               dep_tracking_offset=50000 + j)
                sc = nc.gpsimd.indirect_dma_start(
                    out=view,
                    out_offset=bass.IndirectOffsetOnAxis(
                        ap=ot[:, N_SW_FILLS + j:N_SW_FILLS + j + 1], axis=0),
                    in_=vals[:, j:j + 1], in_offset=None,
                    bounds_check=OUT_ELEMS - 1, oob_is_err=False)
                if j < 3:
                    bmax = _col_bmax(j)
                    deps = sw[1:1 + N_SH_ONES] + covering(bmax, 0)
                else:
                    bmax = ZBASE + _col_bmax(j - 3)
                    deps = [sw[0]] + sw[1 + N_SH_ONES:] + covering(bmax, ZBASE)
                for f in deps:
                    add_dep_helper(sc.ins, f.ins,
                                   reason="scatter after covering fills")

    nc.compile()
    return nc


def _compute_indices(coord_v, lows, highs, nmc, L_):
    """Replicates reference.py exactly (same jax ops on the default device)
    so the floor/log10 bin boundaries match bit-for-bit."""
    import jax.numpy as jnp

    cv = jnp.asarray(np.asarray(coord_v, dtype=np.float32))
    n = cv.shape[1] // 3
    v10 = cv.at[:, 2::3].set(jnp.log10(cv[:, 2::3]))
    lo = jnp.tile(jnp.asarray(np.asarray(lows, dtype=np.float32)), n)
    hi = jnp.tile(jnp.asarray(np.asarray(highs, dtype=np.float32)), n)
    coord_grid = (v10 - lo) / (hi - lo)
    tr = coord_grid.reshape(-1, 3)
    x_i = jnp.floor(tr[:, 0] * L_).astype(jnp.int32)
    y_i = jnp.floor(tr[:, 1] * L_).astype(jnp.int32)
    m_i = jnp.floor(tr[:, 2] * nmc).astype(jnp.int32)
    return (np.asarray(x_i), np.asarray(y_i), np.asarray(m_i))


def _prepare_in_maps(coord_v, lows, highs, nmc, L):
    nmc = int(nmc)
    L_ = int(L)
    x_i, y_i, m_i = _compute_indices(coord_v, lows, highs, nmc, L_)
    n_batch = coord_v.shape[0]
    n = coord_v.shape[1] // 3
    b_i = np.repeat(np.arange(n_batch, dtype=np.int64), n)

    s_local = (b_i % BL).astype(np.int64)
    off_in_half = (m_i.astype(np.int64) * PLANE
                   + y_i.astype(np.int64) * L_ + x_i.astype(np.int64))
    ones_off = s_local * HALF + off_in_half
    zeros_off = REGION + ones_off

    in_maps = []
    pts_per_core = BL * n  # 320
    for c in range(NCORES):
        oc, zc = _LAYOUTS[STARVE_POS.get(c)]
        offs_np = np.full((128, NCOL), OOB, dtype=np.int32)
        # zeros head: 4KB units covering blocks HEAD_START..+15
        offs_np[:, 0] = (HEAD_START * (BLK // HEAD_UNIT)
                         + np.arange(128, dtype=np.int32))
        m = oc >= 0
        offs_np[:, 1:1 + N_SH_ONES][m] = oc[m].astype(np.int32)
        m = zc >= 0
        offs_np[:, 1 + N_SH_ONES:N_SW_FILLS][m] = zc[m].astype(np.int32)

        sel = slice(c * pts_per_core, (c + 1) * pts_per_core)
        po = np.sort(ones_off[sel])
        pz = np.sort(zeros_off[sel])
        for j in range(3):
            colp = po[128 * j:128 * j + 128]
            col = np.full(128, po[0], dtype=np.int64)
            col[:len(colp)] = colp
            offs_np[:, N_SW_FILLS + j] = col.astype(np.int32)
            assert (colp // BLK).max(initial=0) <= _col_bmax(j)
            colz = pz[128 * j:128 * j + 128]
            col = np.full(128, pz[0], dtype=np.int64)
            col[:len(colz)] = colz
            offs_np[:, N_SW_FILLS + 3 + j] = col.astype(np.int32)
            assert (colz // BLK).max(initial=0) <= ZBASE + _col_bmax(j)
        in_maps.append({"offs": offs_np})
    return in_maps


def _run(in_maps, **kwargs):
    if "nc" not in _CACHE:
        _CACHE["nc"] = _build_nc()
    nc = _CACHE["nc"]
    from concourse.bass_utils import run_bass_kernel_spmd
    return run_bass_kernel_spmd(nc, in_maps, core_ids=list(range(NCORES)),
                                **kwargs)


def kernel(coord_v, lows, highs, nmc, L):
    nmc = int(nmc)
    L_ = int(L)
    assert nmc == NMC and L_ == globals()["L"], (nmc, L_)

    in_maps = _prepare_in_maps(coord_v, lows, highs, nmc, L_)
    res = _run(in_maps)
    parts = []
    for c in range(NCORES):
        o = res.results[c]["out"]
        ones = o[0:REGION].reshape(BL, NMC, L_, L_)
        zeros = o[REGION:].reshape(BL, NMC, L_, L_)
        parts.append(np.concatenate((ones, zeros), axis=1))
    return np.concatenate(parts, axis=0)
